# revision 4
# baseline (speedup 1.0000x reference)
"""Relation-aware attention alignment kernel for 8 TRN2 NeuronCores.

Computes m2c = softmax((q @ kc.T + gather(p, rel_c)) / sqrt(H)) and the
analogous m2t, where p = q @ rel_k_emb.T, q = enc @ Wq + bq, k* = {c,t} @ Wk
+ bk, and gather(p, rel)[i, j] = p[i, rel[i, j]].

Sharding: rows of the L=1024 memory axis are split 128 per core; the small
projection weights, key sequences and relation-embedding table are
replicated. Softmax is row-wise so cores never communicate.

Per-core algorithm (all layouts transposed so the contraction dim sits on
SBUF partitions):
  qT   = Wq.T-contract(encT) + bq, scaled by 1/sqrt(H)      [H, 128] PE
  kckt = Wk-contract([cT | tT]) + bk                        [H, 512] PE
  p    = qT.T @ embT                                        [128, 51] PE
  S    = qT.T @ kckt (start) then for each relation r:
           S += I.T @ ((rel == r) * p[:, r])                PE accumulate,
         with the bf16 mask tiles built on the Vector engine (4x mode)
  m2c/m2t = row softmax of S's column blocks                 ACT exp + DVE
"""

import math
import sys
import types

import numpy as np

import concourse.bass as bass
import concourse.tile as tile
from concourse import mybir
from concourse.bass_utils import run_bass_kernel_spmd
from concourse.vector_clock import ScopedClock

try:
    import ml_dtypes

    _BF16 = ml_dtypes.bfloat16
except ImportError:  # pragma: no cover
    _BF16 = None

H = 128
NUM_REL = 51
LQ, LC, LT = 512, 384, 128
L = LQ + LC + LT
LK = LC + LT  # 512 score columns per row
N_CORES = 8
ROWS = L // N_CORES  # 128 rows per core
SCALE = 1.0 / math.sqrt(H)

FP32 = mybir.dt.float32
BF16 = mybir.dt.bfloat16


# ---------------------------------------------------------------------------
# Environment patches: this walrus build accepts at most ONE sync wait per
# instruction, but Tile's kernel-tail drain accumulates one wait per logical
# processor. Split the waits across standalone drain instructions, and skip
# the two all-engine barriers around the semaphore clears (the NRT epilogue
# already quiesces the engines; the range clears keep re-execution safe).
# ---------------------------------------------------------------------------
_ORIG_DRAIN_AND_BARRIER = tile.TileContext._drain_and_barrier
_FOR_SIM = False  # set True to build a CoreSim-compatible graph


def _patched_drain_and_barrier(self, tick_clock, wait_clock):
    if _FOR_SIM:
        return _ORIG_DRAIN_AND_BARRIER(self, tick_clock, wait_clock)
    nc = self.nc
    drain_inst = nc.sync.drain()
    wait_clock.add_sem_waits(
        drain_inst.ins, ScopedClock({None: tick_clock.global_clock})
    )
    si = drain_inst.ins.sync_info
    waits = list(si.on_wait or [])
    if len(waits) > 1:
        si.on_wait = waits[:1]
        for w in waits[1:]:
            extra = nc.sync.drain()
            extra.ins.sync_info = mybir.SyncInfo(on_wait=[w], on_update=[])
    nc.all_engine_barrier()
    popped = nc._tile_sem_poison_stack.pop()
    assert popped is self._sem_poison
    nc.clear_and_free_semaphores(list(self.sems.allocated().values()))


tile.TileContext._drain_and_barrier = _patched_drain_and_barrier


def _split_multi_waits(nc):
    """Safety net: splice extra wait-carrier drains before any instruction
    that still carries more than one sync wait."""
    for fn in nc.m.functions:
        stack = list(fn.blocks)
        while stack:
            bb = stack.pop()
            changed = False
            new_insts = []
            for inst in bb.instructions:
                for b in getattr(inst, "blocks", []) or []:
                    stack.append(b)
                si = inst.sync_info
                if si is not None and si.on_wait and len(si.on_wait) > 1:
                    waits = list(si.on_wait)
                    si.on_wait = waits[-1:]
                    for j, w in enumerate(waits[:-1]):
                        carrier = mybir.InstDrain(
                            name=f"{inst.name}-wsplit{j}", ins=[], outs=[]
                        )
                        carrier.engine = inst.engine
                        carrier.sync_info = mybir.SyncInfo(
                            on_wait=[w], on_update=[]
                        )
                        new_insts.append(carrier)
                    changed = True
                new_insts.append(inst)
            if changed:
                bb.instructions = new_insts


def _install_ntff_hook():
    """Register the axon NTFF profiling hook if this image's antenv lacks
    `axon_hooks` (lets run_bass_kernel_spmd(trace=True) report exec time)."""
    try:
        import antenv.axon_hooks  # noqa: F401

        return
    except ImportError:
        pass
    try:
        import antenv
        from trn_agent_boot.trn_boot import _ntff_profile_via_ctypes
    except ImportError:
        return
    mod = types.ModuleType("antenv.axon_hooks")
    _hook = [None]
    mod.set_axon_ntff_profile_hook = lambda h: _hook.__setitem__(0, h)
    mod.get_axon_ntff_profile_hook = lambda: _hook[0]
    sys.modules["antenv.axon_hooks"] = mod
    antenv.axon_hooks = mod
    try:
        h = _ntff_profile_via_ctypes("/opt/axon/libaxon_pjrt.so")
        if h is not None:
            mod.set_axon_ntff_profile_hook(h)
    except Exception:
        pass


_install_ntff_hook()


# ---------------------------------------------------------------------------
# Bass graph (SPMD: one graph, per-core inputs differ)
# ---------------------------------------------------------------------------
def _build():
    nc = bass.Bass()

    encT_ext = nc.declare_dram_parameter("encT", [H, ROWS], FP32, isOutput=False)
    ctT_ext = nc.declare_dram_parameter("ctT", [H, LK], FP32, isOutput=False)
    wq_ext = nc.declare_dram_parameter("Wq", [H, H], FP32, isOutput=False)
    wk_ext = nc.declare_dram_parameter("Wk", [H, H], FP32, isOutput=False)
    bq_ext = nc.declare_dram_parameter("bq", [H, 1], FP32, isOutput=False)
    bk_ext = nc.declare_dram_parameter("bk", [H, 1], FP32, isOutput=False)
    embT_ext = nc.declare_dram_parameter("embT", [H, NUM_REL], FP32, isOutput=False)
    rel_ext = nc.declare_dram_parameter("rel", [ROWS, LK], BF16, isOutput=False)
    ident_ext = nc.declare_dram_parameter("ident", [H, H], BF16, isOutput=False)
    m2c_ext = nc.declare_dram_parameter("m2c", [ROWS, LC], FP32, isOutput=True)
    m2t_ext = nc.declare_dram_parameter("m2t", [ROWS, LT], FP32, isOutput=True)

    with tile.TileContext(nc) as tc:
        with (
            tc.tile_pool(name="consts", bufs=1) as consts,
            tc.tile_pool(name="work", bufs=1) as work,
            tc.tile_pool(name="masks", bufs=4) as masks,
            tc.tile_pool(name="psA", bufs=1, space="PSUM") as psA,
            tc.tile_pool(name="psB", bufs=1, space="PSUM") as psB,
            tc.tile_pool(name="psS", bufs=1, space="PSUM") as psS,
        ):
            # ---- loads -------------------------------------------------
            encT_sb = consts.tile([H, ROWS], FP32, tag="encT")
            nc.sync.dma_start(encT_sb[:], encT_ext[:])
            ctT_sb = consts.tile([H, LK], FP32, tag="ctT")
            nc.sync.dma_start(ctT_sb[:], ctT_ext[:])
            wq_sb = consts.tile([H, H], FP32, tag="wq")
            nc.sync.dma_start(wq_sb[:], wq_ext[:])
            wk_sb = consts.tile([H, H], FP32, tag="wk")
            nc.sync.dma_start(wk_sb[:], wk_ext[:])
            bq_sb = consts.tile([H, 1], FP32, tag="bq")
            nc.sync.dma_start(bq_sb[:], bq_ext[:])
            bk_sb = consts.tile([H, 1], FP32, tag="bk")
            nc.sync.dma_start(bk_sb[:], bk_ext[:])
            embT_sb = consts.tile([H, NUM_REL], FP32, tag="embT")
            nc.sync.dma_start(embT_sb[:], embT_ext[:])
            rel_sb = consts.tile([ROWS, LK], BF16, tag="rel")
            nc.sync.dma_start(rel_sb[:], rel_ext[:])
            ident_sb = consts.tile([H, H], BF16, tag="ident")
            nc.sync.dma_start(ident_sb[:], ident_ext[:])

            # Warm the ACT exp table early so the ~1.3us table load overlaps
            # the projection matmuls instead of stalling the softmax.
            warm = work.tile([128, 1], FP32, tag="warm")
            nc.scalar.activation(
                warm[:], bq_sb[:], mybir.ActivationFunctionType.Exp
            )

            # ---- projections (all transposed: contraction on partitions)
            qT_ps = psA.tile([H, ROWS], FP32, tag="qT_ps")
            nc.tensor.matmul(qT_ps[:], lhsT=wq_sb[:], rhs=encT_sb[:])
            # qT = (qT + bq) * scale; also build a bf16 copy for the scores
            qT_sb = work.tile([H, ROWS], FP32, tag="qT")
            nc.vector.tensor_scalar(
                qT_sb[:], qT_ps[:], bq_sb[:], SCALE,
                mybir.AluOpType.add, mybir.AluOpType.mult,
            )

            kckt_ps = psB.tile([H, LK], FP32, tag="kckt_ps")
            nc.tensor.matmul(kckt_ps[:, 0:LC], lhsT=wk_sb[:], rhs=ctT_sb[:, 0:LC])
            nc.tensor.matmul(kckt_ps[:, LC:LK], lhsT=wk_sb[:], rhs=ctT_sb[:, LC:LK])
            kckt_sb = work.tile([H, LK], FP32, tag="kckt")
            nc.vector.tensor_scalar(
                kckt_sb[:], kckt_ps[:], bk_sb[:], None, mybir.AluOpType.add
            )

            # ---- p = qT.T @ embT  (relation logits, pre-scaled) ---------
            p_ps = psA.tile([ROWS, NUM_REL], FP32, tag="p_ps")
            nc.tensor.matmul(p_ps[:], lhsT=qT_sb[:], rhs=embT_sb[:])
            p_sb = work.tile([ROWS, NUM_REL], FP32, tag="p")
            nc.vector.tensor_copy(p_sb[:], p_ps[:])

            # ---- scores: base + relation one-hot accumulation ----------
            s_ps = psS.tile([ROWS, LK], FP32, tag="scores")
            nc.tensor.matmul(
                s_ps[:], lhsT=qT_sb[:], rhs=kckt_sb[:], start=True, stop=False
            )
            for r in range(NUM_REL):
                mask = masks.tile([ROWS, LK], BF16, tag="mask")
                nc.vector.tensor_scalar(
                    mask[:], rel_sb[:], float(r), p_sb[:, r : r + 1],
                    mybir.AluOpType.is_equal, mybir.AluOpType.mult,
                )
                nc.tensor.matmul(
                    s_ps[:], lhsT=ident_sb[:], rhs=mask[:],
                    start=False, stop=(r == NUM_REL - 1),
                )

            # ---- row softmax per block ---------------------------------
            for name, lo, hi, out_ext in (
                ("c", 0, LC, m2c_ext),
                ("t", LC, LK, m2t_ext),
            ):
                n = hi - lo
                nmax = work.tile([ROWS, 1], FP32, tag=f"nmax_{name}")
                nc.vector.tensor_reduce(
                    nmax[:], s_ps[:, lo:hi], mybir.AxisListType.X,
                    mybir.AluOpType.max, negate=True,
                )
                e_sb = work.tile([ROWS, n], FP32, tag=f"e_{name}")
                rowsum = work.tile([ROWS, 1], FP32, tag=f"sum_{name}")
                nc.scalar.activation(
                    e_sb[:], s_ps[:, lo:hi], mybir.ActivationFunctionType.Exp,
                    bias=nmax[:], scale=1.0, accum_out=rowsum[:],
                )
                rec = work.tile([ROWS, 1], FP32, tag=f"rec_{name}")
                nc.vector.reciprocal(rec[:], rowsum[:])
                out_sb = work.tile([ROWS, n], FP32, tag=f"out_{name}")
                nc.vector.tensor_scalar(
                    out_sb[:], e_sb[:], rec[:], None, mybir.AluOpType.mult
                )
                nc.sync.dma_start(out_ext[:], out_sb[:])

    if not _FOR_SIM:
        _split_multi_waits(nc)
    return nc


_NC_CACHE = None


def _get_nc():
    global _NC_CACHE
    if _NC_CACHE is None:
        _NC_CACHE = _build()
    return _NC_CACHE


# ---------------------------------------------------------------------------
# Host entry point
# ---------------------------------------------------------------------------
def kernel(
    desc=None,
    q_enc=None,
    c_enc=None,
    t_enc=None,
    relations=None,
    Wq=None,
    bq=None,
    Wk=None,
    bk=None,
    rel_k_emb=None,
    _trace=False,
    _tmpdir=None,
):
    f32 = np.float32
    enc = np.concatenate(
        (np.asarray(q_enc), np.asarray(c_enc), np.asarray(t_enc)), axis=1
    )[0].astype(f32)  # [L, H]
    c = np.asarray(c_enc)[0].astype(f32)
    t = np.asarray(t_enc)[0].astype(f32)
    ctT = np.ascontiguousarray(np.concatenate((c, t), axis=0).T)  # [H, LK]
    Wq_ = np.ascontiguousarray(np.asarray(Wq).astype(f32))
    Wk_ = np.ascontiguousarray(np.asarray(Wk).astype(f32))
    bq_ = np.asarray(bq).astype(f32).reshape(H, 1)
    bk_ = np.asarray(bk).astype(f32).reshape(H, 1)
    embT = np.ascontiguousarray(np.asarray(rel_k_emb).astype(f32).T)  # [H, R]
    rel = np.asarray(relations)[:, LQ:]  # [L, LK] ids in [0, NUM_REL)
    rel_bf16 = rel.astype(_BF16)
    ident = np.eye(H, dtype=_BF16)

    shared = {
        "ctT": ctT,
        "Wq": Wq_,
        "Wk": Wk_,
        "bq": bq_,
        "bk": bk_,
        "embT": embT,
        "ident": ident,
    }
    in_maps = []
    for core in range(N_CORES):
        rows = slice(core * ROWS, (core + 1) * ROWS)
        m = dict(shared)
        m["encT"] = np.ascontiguousarray(enc[rows].T)
        m["rel"] = np.ascontiguousarray(rel_bf16[rows])
        in_maps.append(m)

    nc = _get_nc()
    res = run_bass_kernel_spmd(
        nc,
        in_maps,
        core_ids=list(range(N_CORES)),
        trace=_trace,
        tmpdir=_tmpdir,
    )
    m2c = np.concatenate([res.results[i]["m2c"] for i in range(N_CORES)], axis=0)
    m2t = np.concatenate([res.results[i]["m2t"] for i in range(N_CORES)], axis=0)
    if _trace:
        kernel.last_exec_time_ns = res.exec_time_ns
    return (m2c, m2t)


kernel.last_exec_time_ns = None
